# revision 8
# baseline (speedup 1.0000x reference)
"""Multi-head attention kernel for Trainium2, 8 NeuronCores.

Sharding: data-parallel over (batch, query-half): core i handles batch i//2
and query rows (i%2)*1024 ... +1024. Each core computes K/V over the full
sequence of its batch (K/V projection duplicated between the 2 cores of a
batch; no collectives), then attention for all 16 heads over its query half,
then the output projection for its query rows.

Per-core dataflow (activations kept "transposed" so the contraction dim sits
on SBUF partitions):
  xT  [1024, 2048]   d_model-major input; this core's query-half columns first
  V   = x @ Wv + bv          -> DRAM scratch [2048, 1024]
  K^T = Wk^T x^T (per pair)  -> DRAM scratch [8][128, 2048]
  Q^T = Wq^T x^T (per pair)  -> DRAM scratch [8][128, 1024]
  per head: scores^T[sk,sq] = K^T-slice.T @ Q^T ; P^T = exp(scores^T/8) (ACT)
            out^T[65, sq] = [V_h | 1].T @ P^T   (row 64 = softmax denominator)
            out^T[0:64] * (1/row64) -> outT
  y = outT.T @ Wo^T + bo  -> [1024, 1024] query-half output rows

All matmul operands are float32r (TF32-like, full PE rate at N>=256);
accumulation is fp32 in PSUM.
"""

import os

os.environ.setdefault("MYCRO_LOCAL_CACHE", "1")

import numpy as np

try:
    import concourse.bass as bass
except ImportError:  # pragma: no cover
    import sys

    for p in ("/opt/trn_rl_repo", "/root/.axon_site/_ro/trn_rl_repo"):
        if os.path.isdir(p) and p not in sys.path:
            sys.path.insert(0, p)
    import concourse.bass as bass

import concourse.mybir as mybir
import concourse.tile as tile
from concourse import bacc, bass_utils

F32R = mybir.dt.float32r
F32 = mybir.dt.float32
AF = mybir.ActivationFunctionType

B = 4
S = 2048
D_MODEL = 1024
H = 16
HD = 64
NPAIR = 8
KT = 8
SQ = 1024
NT = S // 128  # 16 sk-tiles
N_CORES = 8

_CACHE: dict = {}


def build_program():
    nc = bacc.Bacc("TRN2", target_bir_lowering=False, debug=False)

    xT = nc.dram_tensor("xT", [D_MODEL, S], F32R, kind="ExternalInput")
    wq = nc.dram_tensor("wq", [D_MODEL, D_MODEL], F32R, kind="ExternalInput")
    wk = nc.dram_tensor("wk", [D_MODEL, D_MODEL], F32R, kind="ExternalInput")
    wv = nc.dram_tensor("wv", [D_MODEL, D_MODEL], F32R, kind="ExternalInput")
    bv = nc.dram_tensor("bv", [1, D_MODEL], F32R, kind="ExternalInput")
    bq = nc.dram_tensor("bq", [NPAIR, 128], F32, kind="ExternalInput")
    bk = nc.dram_tensor("bk", [NPAIR, 128], F32, kind="ExternalInput")
    wo = nc.dram_tensor("wo", [D_MODEL, D_MODEL], F32R, kind="ExternalInput")
    bo = nc.dram_tensor("bo", [1, D_MODEL], F32R, kind="ExternalInput")
    ones_in = nc.dram_tensor("ones_in", [1, 128], F32R, kind="ExternalInput")
    ones_tk = nc.dram_tensor("ones_tk", [128, NT], F32R, kind="ExternalInput")
    y = nc.dram_tensor("y", [SQ, D_MODEL], F32, kind="ExternalOutput")

    with tile.TileContext(nc) as tc:
        with (
            tc.tile_pool(name="pers", bufs=1) as pers,
            tc.tile_pool(name="dram", bufs=1, space="DRAM") as dram,
        ):
            outT = pers.tile([128, NPAIR, SQ], F32R)  # normalized out^T, 4 MiB
            ones_sb = pers.tile([1, 128], F32R)
            bv_sb = pers.tile([1, D_MODEL], F32R)
            bo_sb = pers.tile([1, D_MODEL], F32R)
            bq_sb = pers.tile([128, NPAIR], F32)
            bk_sb = pers.tile([128, NPAIR], F32)
            nc.sync.dma_start(ones_sb[:], ones_in.ap())
            nc.sync.dma_start(bv_sb[:], bv.ap())
            nc.sync.dma_start(bo_sb[:], bo.ap())
            nc.sync.dma_start(bq_sb[:], bq.ap().rearrange("p r -> r p"))
            nc.sync.dma_start(bk_sb[:], bk.ap().rearrange("p r -> r p"))

            vscr = dram.tile([S, D_MODEL], F32R)
            qscr = dram.tile([NPAIR, 128, SQ], F32R)
            kscr = dram.tile([NPAIR, 128, S], F32R)

            # attention streaming pools (opened early so prefetches can be
            # emitted from inside the KQ phase)
            with (
                tc.tile_pool(name="vst", bufs=2) as vsp,
                tc.tile_pool(name="ktp", bufs=2) as ktp,
                tc.tile_pool(name="qtp", bufs=2) as qtp,
            ):
                vp_t, kt_t, qt_t = {}, {}, {}

                def prefetch_pair(p):
                    vp_sb = vsp.tile([128, NT, 130], F32R, tag="vp", name=f"vp{p}")
                    for a in range(2):
                        nc.sync.dma_start(
                            vp_sb[:, :, a * 65 : a * 65 + 64],
                            vscr[:, 128 * p + a * 64 : 128 * p + (a + 1) * 64].rearrange(
                                "(t r) c -> r t c", r=128
                            ),
                        )
                        nc.sync.dma_start(
                            vp_sb[:, :, a * 65 + 64 : a * 65 + 65],
                            ones_tk.ap().rearrange("r (t o) -> r t o", o=1),
                        )
                    vp_t[p] = vp_sb
                    kt_sb = ktp.tile([128, S], F32R, tag="kt", name=f"kt{p}")
                    nc.sync.dma_start(kt_sb[:], kscr[p])
                    kt_t[p] = kt_sb
                    qt_sb = qtp.tile([128, SQ], F32R, tag="qt", name=f"qt{p}")
                    nc.sync.dma_start(qt_sb[:], qscr[p])
                    qt_t[p] = qt_sb

                # ---------------- phase V + KQ (xT resident) ----------------
                with tc.tile_pool(name="xp", bufs=1) as xp:
                    xt_sb = xp.tile([128, KT, S], F32R)
                    with (
                        tc.tile_pool(name="vw", bufs=2) as vwp,
                        tc.tile_pool(name="vps", bufs=3, space="PSUM") as vpsp,
                        tc.tile_pool(name="vd", bufs=3) as vdp,
                    ):
                        # first V weight block, then xT k-tiles (so compute can
                        # start as soon as the needed slices land)
                        wv_sbs = []
                        for i, c0 in enumerate((0, 512)):
                            wv_sb = vwp.tile([128, KT, 512], F32R, tag="wv", name=f"wv{c0}")
                            nc.sync.dma_start(
                                wv_sb[:],
                                wv.ap()[:, c0 : c0 + 512].rearrange(
                                    "(k r) c -> r k c", r=128
                                ),
                            )
                            wv_sbs.append(wv_sb)
                            if i == 0:
                                for k in range(KT):
                                    nc.sync.dma_start(
                                        xt_sb[:, k, :],
                                        xT.ap()[128 * k : 128 * (k + 1), :],
                                    )
                        for i, c0 in enumerate((0, 512)):
                            wv_sb = wv_sbs[i]
                            for t in range(NT):
                                vps = vpsp.tile(
                                    [128, 512], F32, tag="vps", name=f"vps{c0}_{t}"
                                )
                                for k in range(KT):
                                    nc.tensor.matmul(
                                        vps[:],
                                        xt_sb[:, k, t * 128 : (t + 1) * 128],
                                        wv_sb[:, k, :],
                                        start=(k == 0),
                                        stop=False,
                                    )
                                nc.tensor.matmul(
                                    vps[:],
                                    ones_sb[:],
                                    bv_sb[:, c0 : c0 + 512],
                                    start=False,
                                    stop=True,
                                )
                                vsb = vdp.tile([128, 512], F32R, tag="vsb", name=f"vsb{c0}_{t}")
                                nc.scalar.activation(vsb[:], vps[:], AF.Copy)
                                nc.sync.dma_start(
                                    vscr[t * 128 : (t + 1) * 128, c0 : c0 + 512], vsb[:]
                                )

                    # K^T and Q^T per head pair -> DRAM scratch
                    with (
                        tc.tile_pool(name="wkq", bufs=2) as wkqp,
                        tc.tile_pool(name="kps", bufs=5, space="PSUM") as kpsp,
                        tc.tile_pool(name="qps", bufs=3, space="PSUM") as qpsp,
                        tc.tile_pool(name="kd", bufs=2) as kdp,
                        tc.tile_pool(name="qd", bufs=2) as qdp,
                    ):
                        for p in range(NPAIR):
                            wk_sb = wkqp.tile([128, KT, 128], F32R, tag="wk", name=f"wk{p}")
                            nc.sync.dma_start(
                                wk_sb[:],
                                wk.ap()[:, 128 * p : 128 * (p + 1)].rearrange(
                                    "(k r) c -> r k c", r=128
                                ),
                            )
                            wq_sb = wkqp.tile([128, KT, 128], F32R, tag="wq", name=f"wq{p}")
                            nc.sync.dma_start(
                                wq_sb[:],
                                wq.ap()[:, 128 * p : 128 * (p + 1)].rearrange(
                                    "(k r) c -> r k c", r=128
                                ),
                            )
                            kps = [
                                kpsp.tile([128, 512], F32, tag="kps", name=f"kps{p}_{j}")
                                for j in range(4)
                            ]
                            qps = [
                                qpsp.tile([128, 512], F32, tag="qps", name=f"qps{p}_{j}")
                                for j in range(2)
                            ]
                            for k in range(KT):
                                for j in range(4):
                                    nc.tensor.matmul(
                                        kps[j][:],
                                        wk_sb[:, k, :],
                                        xt_sb[:, k, j * 512 : (j + 1) * 512],
                                        start=(k == 0),
                                        stop=(k == KT - 1),
                                    )
                                for j in range(2):
                                    nc.tensor.matmul(
                                        qps[j][:],
                                        wq_sb[:, k, :],
                                        xt_sb[:, k, j * 512 : (j + 1) * 512],
                                        start=(k == 0),
                                        stop=(k == KT - 1),
                                    )
                            ksb = kdp.tile([128, S], F32R, tag="ksb", name=f"ksb{p}")
                            for j in range(4):
                                nc.vector.tensor_scalar_add(
                                    ksb[:, j * 512 : (j + 1) * 512],
                                    kps[j][:],
                                    bk_sb[:, p : p + 1],
                                )
                            nc.sync.dma_start(kscr[p], ksb[:])
                            qsb = qdp.tile([128, SQ], F32R, tag="qsb", name=f"qsb{p}")
                            for j in range(2):
                                nc.vector.tensor_scalar_add(
                                    qsb[:, j * 512 : (j + 1) * 512],
                                    qps[j][:],
                                    bq_sb[:, p : p + 1],
                                )
                            nc.sync.dma_start(qscr[p], qsb[:])
                            if p == NPAIR - 1:
                                prefetch_pair(0)

                # ---------------- attention + wo prefetch ----------------
                with tc.tile_pool(name="wop", bufs=1) as wop:
                    wo_sb = wop.tile([128, KT, D_MODEL], F32R)
                    nc.sync.dma_start(
                        wo_sb[:], wo.ap().rearrange("(k r) c -> r k c", r=128)
                    )
                    attn_pools = (
                        tc.tile_pool(name="pt", bufs=5),
                        tc.tile_pool(name="scp", bufs=2, space="PSUM"),
                        tc.tile_pool(name="avp", bufs=2, space="PSUM"),
                        tc.tile_pool(name="sm", bufs=2),
                        tc.tile_pool(name="ntp", bufs=2),
                    )
                    import contextlib
                    _stk = contextlib.ExitStack()
                    ptp, scp, avp, smp, ntp = (_stk.enter_context(pl) for pl in attn_pools)
                    for p in range(NPAIR):
                        if p + 1 < NPAIR:
                            prefetch_pair(p + 1)
                        vp_sb, kt_sb, qt_sb = vp_t.pop(p), kt_t.pop(p), qt_t.pop(p)
                        for a in range(2):
                            av = avp.tile([65, SQ], F32, tag="av", name=f"av{p}_{a}")
                            for t in range(NT):
                                sc = scp.tile(
                                    [128, SQ], F32, tag="sc", name=f"sc{p}_{a}_{t}"
                                )
                                for j in range(2):
                                    nc.tensor.matmul(
                                        sc[:, j * 512 : (j + 1) * 512],
                                        kt_sb[a * 64 : (a + 1) * 64, t * 128 : (t + 1) * 128],
                                        qt_sb[a * 64 : (a + 1) * 64, j * 512 : (j + 1) * 512],
                                        start=True,
                                        stop=True,
                                    )
                                pt = ptp.tile(
                                    [128, SQ], F32R, tag="pt", name=f"pt{p}_{a}_{t}"
                                )
                                nc.scalar.activation(pt[:], sc[:], AF.Exp, scale=0.125)
                                for j in range(2):
                                    nc.tensor.matmul(
                                        av[:, j * 512 : (j + 1) * 512],
                                        vp_sb[:, t, a * 65 : (a + 1) * 65],
                                        pt[:, j * 512 : (j + 1) * 512],
                                        start=(t == 0),
                                        stop=(t == NT - 1),
                                    )
                            rc = smp.tile([128, SQ], F32, tag="rc", name=f"rc{p}_{a}")
                            nc.vector.reciprocal(rc[64:65, :], av[64:65, :])
                            rz = smp.tile([1, SQ], F32, tag="rz", name=f"rz{p}_{a}")
                            nc.sync.dma_start(rz[:], rc[64:65, :])
                            bc = smp.tile([64, SQ], F32, tag="bc", name=f"bc{p}_{a}")
                            nc.gpsimd.partition_broadcast(bc[:], rz[:])
                            nt = ntp.tile([64, SQ], F32R, tag="nt", name=f"nt{p}_{a}")
                            nc.vector.tensor_mul(nt[:], av[0:64, :], bc[:])
                            nc.sync.dma_start(outT[a * 64 : (a + 1) * 64, p, :], nt[:])

                    _stk.close()

                    # ---------------- output projection ----------------
                    with (
                        tc.tile_pool(name="yps", bufs=3, space="PSUM") as ypsp,
                        tc.tile_pool(name="yd", bufs=3) as ydp,
                    ):
                        for m in range(SQ // 128):
                            for nb in range(2):
                                yp = ypsp.tile(
                                    [128, 512], F32, tag="yp", name=f"yp{m}_{nb}"
                                )
                                for k in range(KT):
                                    nc.tensor.matmul(
                                        yp[:],
                                        outT[:, k, m * 128 : (m + 1) * 128],
                                        wo_sb[:, k, nb * 512 : (nb + 1) * 512],
                                        start=(k == 0),
                                        stop=False,
                                    )
                                nc.tensor.matmul(
                                    yp[:],
                                    ones_sb[:],
                                    bo_sb[:, nb * 512 : (nb + 1) * 512],
                                    start=False,
                                    stop=True,
                                )
                                ysb = ydp.tile(
                                    [128, 512], F32, tag="ysb", name=f"ysb{m}_{nb}"
                                )
                                nc.vector.tensor_copy(ysb[:], yp[:])
                                nc.sync.dma_start(
                                    y.ap()[
                                        m * 128 : (m + 1) * 128,
                                        nb * 512 : (nb + 1) * 512,
                                    ],
                                    ysb[:],
                                )

    nc.compile()
    return nc


def prep_inputs(x, Wq, bq, Wk, bk, Wv, bv, Wo, bo):
    """Host-side sharding: returns per-core input maps (numpy only)."""
    x = np.asarray(x, dtype=np.float32)
    Wq = np.asarray(Wq, dtype=np.float32)
    Wk = np.asarray(Wk, dtype=np.float32)
    Wv = np.asarray(Wv, dtype=np.float32)
    Wo = np.asarray(Wo, dtype=np.float32)
    bq = np.asarray(bq, dtype=np.float32)
    bk = np.asarray(bk, dtype=np.float32)
    bv = np.asarray(bv, dtype=np.float32)
    bo = np.asarray(bo, dtype=np.float32)

    shared = {
        "wq": np.ascontiguousarray(Wq.transpose(1, 0, 2).reshape(D_MODEL, D_MODEL)),
        "wk": np.ascontiguousarray(Wk.transpose(1, 0, 2).reshape(D_MODEL, D_MODEL)),
        "wv": np.ascontiguousarray(Wv.transpose(1, 0, 2).reshape(D_MODEL, D_MODEL)),
        "bv": bv.reshape(1, D_MODEL).copy(),
        "bq": np.ascontiguousarray(bq.reshape(NPAIR, 128)),
        "bk": np.ascontiguousarray(bk.reshape(NPAIR, 128)),
        "wo": np.ascontiguousarray(Wo.T),
        "bo": bo.reshape(1, D_MODEL).copy(),
        "ones_in": np.ones((1, 128), dtype=np.float32),
        "ones_tk": np.ones((128, NT), dtype=np.float32),
    }
    in_maps = []
    for core in range(N_CORES):
        b, half = divmod(core, 2)
        xt = x[b].T
        if half == 0:
            xt_core = xt
        else:
            xt_core = np.concatenate([xt[:, SQ:], xt[:, :SQ]], axis=1)
        in_maps.append({"xT": np.ascontiguousarray(xt_core), **shared})
    return in_maps


def assemble_output(results):
    y = np.empty((B, S, D_MODEL), dtype=np.float32)
    for core in range(N_CORES):
        b, half = divmod(core, 2)
        y[b, half * SQ : (half + 1) * SQ, :] = results[core]["y"]
    return y


def _get_runner():
    """Build the program + jitted 8-core executor once; reuse across calls."""
    if "runner" in _CACHE:
        return _CACHE["runner"]

    import jax
    import concourse.mybir as mb
    from concourse import bass2jax
    from jax.sharding import Mesh, PartitionSpec
    from jax.experimental.shard_map import shard_map

    nc = build_program()
    _CACHE["nc"] = nc
    bass2jax.install_neuronx_cc_hook()

    partition_name = (
        nc.partition_id_tensor.name if nc.partition_id_tensor is not None else None
    )
    in_names, out_names, out_avals = [], [], []
    for alloc in nc.m.functions[0].allocations:
        if not isinstance(alloc, mb.MemoryLocationSet):
            continue
        name = alloc.memorylocations[0].name
        if alloc.kind == "ExternalInput":
            if name != partition_name:
                in_names.append(name)
        elif alloc.kind == "ExternalOutput":
            out_names.append(name)
            out_avals.append(
                jax.core.ShapedArray(tuple(alloc.tensor_shape), mb.dt.np(alloc.dtype))
            )
    n_params = len(in_names)
    n_outs = len(out_avals)
    all_in_names = in_names + out_names
    if partition_name is not None:
        all_in_names = all_in_names + [partition_name]

    def _body(*args):
        operands = list(args)
        if partition_name is not None:
            operands.append(bass2jax.partition_id_tensor())
        outs = bass2jax._bass_exec_p.bind(
            *operands,
            out_avals=tuple(out_avals),
            in_names=tuple(all_in_names),
            out_names=tuple(out_names),
            lowering_input_output_aliases=(),
            sim_require_finite=True,
            sim_require_nnan=True,
            nc=nc,
        )
        return tuple(outs)

    devices = jax.devices()[:N_CORES]
    mesh = Mesh(np.asarray(devices), ("core",))
    donate = tuple(range(n_params, n_params + n_outs))
    sharded = jax.jit(
        shard_map(
            _body,
            mesh=mesh,
            in_specs=(PartitionSpec("core"),) * (n_params + n_outs),
            out_specs=(PartitionSpec("core"),) * n_outs,
            check_rep=False,
        ),
        donate_argnums=donate,
        keep_unused=True,
    )

    def run(in_maps):
        concat_in = [
            np.concatenate([np.asarray(m[nm]) for m in in_maps], axis=0)
            for nm in in_names
        ]
        concat_zeros = [
            np.zeros((N_CORES * a.shape[0], *a.shape[1:]), a.dtype) for a in out_avals
        ]
        out_arrs = sharded(*concat_in, *concat_zeros)
        return [
            {
                nm: np.asarray(out_arrs[i]).reshape(N_CORES, *out_avals[i].shape)[c]
                for i, nm in enumerate(out_names)
            }
            for c in range(N_CORES)
        ]

    _CACHE["runner"] = run
    return run


def kernel(**inputs):
    run = _get_runner()
    in_maps = prep_inputs(**inputs)
    return assemble_output(run(in_maps))
